# revision 8
# baseline (speedup 1.0000x reference)
"""CapsuleModel2 segment-reduce kernel for 8 TRN2 NeuronCores (v2: ap_gather).

Math (per reference.py):
    feats  = class_capsules.reshape(65536, 272)[point_idx]        # [P, 272]
    sums   = segment_sum(feats, segment_ids, 4096)                # [4096, 272]
    counts = segment_sum(ones)                                    # [4096]
    out    = sigmoid((sums / max(counts,1)) @ W + b)              # [4096, 19]

Key identity: (sums @ W) = segment_sum(feats @ W) — project the grid down to
19 channels FIRST, then reduce per segment.

v1 (baseline) gathered per-point rows of a DRAM table via SWDGE dma_gather:
73728 descriptors x 256B per core; descriptor GENERATION on the Q7 cores
(~8ns/desc) dominated the 530us runtime.

v2 keeps the projected table IN SBUF (channels on partitions) and fetches
point rows with gpsimd.ap_gather — a pure SBUF-local free-dim gather, no DMA
descriptors at all:
  - table[ch, cell]: 19 rows x 8192 cells f32, +1 zero col for padding,
    replicated on 4 partition blocks (32*b..32*b+18) so all 8 Q7 cores work
  - the core's points are split into 4 streams (window w -> stream w%4),
    each stream gathered by its block: out[32b+ch, slot] = table[ch, cell]
  - per 128-slot chunk: PE-transpose [19,128] -> [128,19], append a ones
    column, then the one-hot segment matmul psum[64,20] += oh^T @ X
    (col 19 accumulates the segment COUNT; padding has segrel=-1 -> oh=0)
  - ONE ReduceScatter(add) gives core k its 512 segments; finalize divides
    by count, un-scales (W was x16 for fp8), bias, sigmoid.

Distribution: core k owns grid rows [k*8192, (k+1)*8192) and the points that
hit them (table-sharded), all 4096 segments partially -> ReduceScatter.
"""

import sys

for _p in ('/opt/trn_rl_repo',):
    if _p not in sys.path:
        sys.path.insert(0, _p)

import numpy as np
import ml_dtypes

import concourse.bacc as bacc
import concourse.bass as bass
import concourse.mybir as mybir
import concourse.tile as tile

BF16 = mybir.dt.bfloat16
F32 = mybir.dt.float32
I16 = mybir.dt.int16
F16 = mybir.dt.float16
F8 = mybir.dt.float8e4

NCORE = 8
GRID = 65536
GPC = GRID // NCORE          # 8192 grid rows (cells) per core
D = 272                      # capsule feature dim
NCH = 19                     # output channels
NW = NCH + 1                 # + count column in the reduce output
NSEG = 4096
WIN = 64                     # segments per window (one-hot width)
NWIN = NSEG // WIN           # 64 windows
CAP = 1152                   # padded points per (core, window); actual max ~1118
CPW = CAP // 128             # 9 chunks per window
NSTREAM = 4                  # ap_gather replica blocks (32 partitions each)
WPS = NWIN // NSTREAM        # 16 windows per stream
SLOTS = WPS * CAP            # 18432 slots per stream
NCALL = 8                    # gather calls (pipeline granularity)
SPC = SLOTS // NCALL         # 2304 slots per stream per call
CPC = SPC // 128             # 18 chunks per stream per call
SEG_PER_CORE = NSEG // NCORE  # 512
NBLK = NSEG // 128           # 32 output blocks (2 windows each)
MT = 512                     # cells per projection matmul
NMT = GPC // MT              # 16
ZCOL = GPC                   # zero-row column index (padding target)
WSCALE = 16.0                # fp8 W pre-scale; undone in finalize


def build_nc(skip_collective=False):
    nc = bacc.Bacc("TRN2", num_devices=NCORE)

    gridT = nc.dram_tensor("gridT", [D, GPC], F8, kind="ExternalInput")
    w_pack = nc.dram_tensor("w_pack", [128, 3 * NCH], F8, kind="ExternalInput")
    idx_in = nc.dram_tensor("idx", [NCALL, 128, SPC // 16], I16,
                            kind="ExternalInput")
    segrel_in = nc.dram_tensor("segrel", [128, NCALL * 72], BF16,
                               kind="ExternalInput")
    iota_in = nc.dram_tensor("iota", [128, WIN], BF16, kind="ExternalInput")
    bias_in = nc.dram_tensor("bias", [128, NCH], F32, kind="ExternalInput")
    eye_in = nc.dram_tensor("eye", [128, 128], BF16, kind="ExternalInput")
    if skip_collective:
        out_t = nc.dram_tensor("out", [NBLK, 128, NW], F16, kind="ExternalOutput")
    else:
        out_t = nc.dram_tensor("out", [SEG_PER_CORE, NCH], F32,
                               kind="ExternalOutput")

    partial_d = nc.dram_tensor("partial", [NBLK, 128, NW], F16)
    rs_out = nc.dram_tensor("rs_out", [NBLK // NCORE, 128, NW], F16)

    ksizes = [(0, 128), (128, 128), (256, 16)]

    with tile.TileContext(nc) as tc:
        with (
            tc.tile_pool(name="const", bufs=1) as cpool,
            tc.tile_pool(name="tabp", bufs=1) as tabpool,
            tc.tile_pool(name="grid", bufs=1) as gpool,
            tc.tile_pool(name="proj", bufs=2, space="PSUM") as prpool,
            tc.tile_pool(name="dst", bufs=2) as dpool,
            tc.tile_pool(name="dstb", bufs=2) as bpool,
            tc.tile_pool(name="gsb", bufs=2) as spool,
            tc.tile_pool(name="oh", bufs=2) as opool,
            tc.tile_pool(name="pt", bufs=3, space="PSUM") as ptpool,
            tc.tile_pool(name="pblk", bufs=3, space="PSUM") as pbpool,
            tc.tile_pool(name="acc", bufs=1) as apool,
            tc.tile_pool(name="fin", bufs=2) as fpool,
        ):
            # constants
            w_sb = cpool.tile([128, 3, NCH], F8)
            nc.sync.dma_start(w_sb[:], w_pack[:].rearrange(
                "p (t c) -> p t c", t=3))
            segrel_sb = cpool.tile([128, NCALL * 72], BF16)
            nc.sync.dma_start(segrel_sb[:], segrel_in[:])
            iota_sb = cpool.tile([128, WIN], BF16)
            nc.sync.dma_start(iota_sb[:], iota_in[:])
            bias_sb = cpool.tile([128, NCH], F32)
            nc.sync.dma_start(bias_sb[:], bias_in[:])
            eye_sb = cpool.tile([128, 128], BF16)
            nc.sync.dma_start(eye_sb[:], eye_in[:])
            idx_sb = cpool.tile([128, NCALL, SPC // 16], I16)
            nc.sync.dma_start(idx_sb[:], idx_in[:].rearrange("s p c -> p s c"))

            # ---- Phase A: transposed projection -> SBUF table ----
            # table[ch, cell] on partitions 0..18 of each 32-partition block
            table = tabpool.tile([128, GPC + 1], F32)
            nc.vector.memset(table[:], 0.0)

            gt = gpool.tile([128, 3, GPC], F8)
            for t, (k0, kn) in enumerate(ksizes):
                nc.sync.dma_start(gt[:kn, t, :], gridT[k0:k0 + kn, :])
            for mt in range(NMT):
                psum = prpool.tile([NCH, MT], F32, tag="proj")
                for t, (k0, kn) in enumerate(ksizes):
                    nc.tensor.matmul(
                        out=psum[:],
                        lhsT=w_sb[:kn, t, :],
                        rhs=gt[:kn, t, mt * MT:(mt + 1) * MT],
                        start=(t == 0), stop=(t == 2))
                nc.scalar.copy(table[:NCH, mt * MT:(mt + 1) * MT], psum[:])
            # replicate to blocks 1..3 (cross-partition -> DMA)
            for b in range(1, NSTREAM):
                nc.sync.dma_start(table[32 * b:32 * b + NCH, :GPC],
                                  table[:NCH, :GPC])

            # ---- Phase B: ap_gather + transpose + one-hot reduce ----
            part_sb = apool.tile([128, NBLK * NW], F16)
            for i in range(NCALL):
                dst = dpool.tile([128, SPC], F32, tag="dst")
                nc.gpsimd.ap_gather(
                    out_ap=dst[:], in_ap=table[:],
                    idxs_ap=idx_sb[:, i, :],
                    channels=128, num_elems=GPC + 1, d=1, num_idxs=SPC)
                dstb = bpool.tile([128, SPC], BF16, tag="dstb")
                nc.vector.tensor_copy(dstb[:], dst[:])

                oh = opool.tile([128, 72, WIN], BF16, tag="oh")
                nc.vector.tensor_tensor(
                    out=oh[:],
                    in0=segrel_sb[:, i * 72:(i + 1) * 72]
                        .rearrange("p (c o) -> p c o", o=1).broadcast_to(
                            [128, 72, WIN]),
                    in1=iota_sb[:].rearrange("p (o j) -> p o j", o=1)
                        .broadcast_to([128, 72, WIN]),
                    op=mybir.AluOpType.is_equal)

                gsb = spool.tile([128, 72, NW], BF16, tag="gsb")
                nc.vector.memset(gsb[:, :, NCH:NW], 1.0)
                # one transpose per stream PAIR: lhsT spans 51 partitions at
                # base 0 (streams 0,1) or 64 (streams 2,3); out cols 0:19 are
                # the even stream's channels, cols 32:51 the odd stream's
                KT = 32 + NCH  # 51
                for b2 in range(2):
                    for c in range(CPC):
                        pt = ptpool.tile([128, KT], BF16, tag="pt")
                        nc.tensor.matmul(
                            out=pt[:],
                            lhsT=dstb[64 * b2:64 * b2 + KT,
                                      c * 128:(c + 1) * 128],
                            rhs=eye_sb[64 * b2:64 * b2 + KT, :KT],
                            is_transpose=True)
                        nc.scalar.copy(gsb[:, (2 * b2) * CPC + c, :NCH],
                                       pt[:, :NCH])
                        nc.scalar.copy(gsb[:, (2 * b2 + 1) * CPC + c, :NCH],
                                       pt[:, 32:KT])

                psum_w = None
                for b in range(NSTREAM):
                    for c in range(CPC):
                        g72 = b * CPC + c
                        w = b + NSTREAM * (2 * i + c // CPW)
                        j = c % CPW
                        if j == 0:
                            psum_w = pbpool.tile([WIN, NW], F32, tag="pblk")
                        nc.tensor.matmul(
                            out=psum_w[:],
                            lhsT=oh[:, g72, :],
                            rhs=gsb[:, g72, :],
                            start=(j == 0), stop=(j == CPW - 1))
                        if j == CPW - 1:
                            blk, half = divmod(w, 2)
                            nc.scalar.copy(
                                part_sb[WIN * half:WIN * half + WIN,
                                        blk * NW:(blk + 1) * NW],
                                psum_w[:])
                # call i completes windows [8i, 8i+8) = blocks [4i, 4i+4)
                nc.sync.dma_start(
                    (out_t if skip_collective else partial_d)
                        [4 * i:4 * i + 4].rearrange("b p c -> p b c"),
                    part_sb[:, 4 * i * NW:(4 * i + 4) * NW]
                        .rearrange("p (b c) -> p b c", b=4))

            if not skip_collective:
                nc.gpsimd.collective_compute(
                    "ReduceScatter",
                    mybir.AluOpType.add,
                    replica_groups=[list(range(NCORE))],
                    ins=[partial_d[:]],
                    outs=[rs_out[:]],
                )

            # ---- Phase C: finalize ----
            if not skip_collective:
                H = NBLK // NCORE  # 4
                fin16 = fpool.tile([128, H, NW], F16, tag="fin16")
                nc.sync.dma_start(fin16[:], rs_out[:].rearrange("h p c -> p h c"))
                fin = fpool.tile([128, H, NW], F32, tag="fin")
                nc.vector.tensor_copy(fin[:], fin16[:])
                cnt = fpool.tile([128, H, 1], F32, tag="cnt")
                nc.vector.tensor_scalar_max(cnt[:], fin[:, :, NCH:NW], 1.0)
                rec = fpool.tile([128, H, 1], F32, tag="rec")
                nc.vector.reciprocal(rec[:], cnt[:])
                sc = fpool.tile([128, H, NCH], F32, tag="sc")
                nc.vector.tensor_tensor(
                    out=sc[:], in0=fin[:, :, :NCH],
                    in1=rec[:].broadcast_to([128, H, NCH]),
                    op=mybir.AluOpType.mult)
                # undo the x16 fp8 W scale, add bias, sigmoid
                sc2 = fpool.tile([128, H, NCH], F32, tag="sc2")
                nc.vector.tensor_scalar_mul(sc2[:], sc[:], 1.0 / WSCALE)
                sc3 = fpool.tile([128, H, NCH], F32, tag="sc3")
                nc.vector.tensor_tensor(
                    out=sc3[:], in0=sc2[:],
                    in1=bias_sb[:].rearrange("p (h c) -> p h c", h=1)
                        .broadcast_to([128, H, NCH]),
                    op=mybir.AluOpType.add)
                og = fpool.tile([128, H, NCH], F32, tag="og")
                nc.scalar.activation(og[:], sc3[:],
                                     mybir.ActivationFunctionType.Sigmoid)
                nc.sync.dma_start(
                    out_t[:].rearrange("(h p) c -> p h c", p=128), og[:])

    nc.compile()
    return nc


def prep_inputs(class_capsules, W, b, point_idx, segment_ids, num_segments=NSEG):
    """Host-side sharding: returns in_maps (list of 8 dicts)."""
    assert int(num_segments) == NSEG
    grid = np.ascontiguousarray(class_capsules.reshape(GRID, D), np.float32)
    point_idx = np.asarray(point_idx, np.int64)
    segment_ids = np.asarray(segment_ids, np.int64)
    W = np.asarray(W, np.float32)
    b = np.asarray(b, np.float32)

    f8 = ml_dtypes.float8_e4m3fn
    w_pack = np.zeros((128, 3 * NCH), f8)
    w16 = (W * WSCALE).astype(f8)
    w_pack[0:128, 0:NCH] = w16[0:128]
    w_pack[0:128, NCH:2 * NCH] = w16[128:256]
    w_pack[0:16, 2 * NCH:3 * NCH] = w16[256:272]

    iota = np.tile(np.arange(WIN, dtype=np.float32), (128, 1)).astype(
        ml_dtypes.bfloat16)
    bias_rep = np.tile(b[None, :], (128, 1)).astype(np.float32)
    # block-periodic identity: eye[p, c] = ((p % 64) == c), so the slices
    # [0:51, :51] and [64:115, :51] are both I_51
    eye = (np.arange(128)[:, None] % 64 ==
           np.arange(128)[None, :]).astype(np.float32).astype(
               ml_dtypes.bfloat16)

    in_maps = []
    for k in range(NCORE):
        sel = (point_idx >= k * GPC) & (point_idx < (k + 1) * GPC)
        lidx = (point_idx[sel] - k * GPC).astype(np.int64)
        lseg = segment_ids[sel]          # sorted ascending
        win = (lseg >> 6).astype(np.int64)
        srel = (lseg & 63).astype(np.float32)
        counts = np.bincount(win, minlength=NWIN)
        assert counts.max() <= CAP, f"core {k}: window count {counts.max()} > CAP"
        start = np.zeros(NWIN, np.int64)
        start[1:] = np.cumsum(counts)[:-1]
        rank = np.arange(lidx.size) - start[win]
        # stream = win % 4; slot within stream: (win//4)*CAP + rank
        stream = win % NSTREAM
        spos = (win // NSTREAM) * CAP + rank

        idx_pad = np.full((NSTREAM, SLOTS), ZCOL, np.int16)   # pad -> zero col
        srel_pad = np.full((NSTREAM, SLOTS), -1.0, np.float32)
        idx_pad[stream, spos] = lidx
        srel_pad[stream, spos] = srel

        # idx wrapped: call i, partition 32b+p (p<16, dup at p+16),
        # col s -> idx_pad[b, i*SPC + s*16 + p]
        idxw = np.zeros((NCALL, 128, SPC // 16), np.int16)
        for i in range(NCALL):
            for bb in range(NSTREAM):
                blk = idx_pad[bb, i * SPC:(i + 1) * SPC].reshape(-1, 16).T
                idxw[i, 32 * bb:32 * bb + 16] = blk
                idxw[i, 32 * bb + 16:32 * bb + 32] = blk

        # segrel columns: i*72 + b*18 + c ; rows: slot within chunk
        segrel_arr = np.full((128, NCALL * 72), -1.0, np.float32)
        for i in range(NCALL):
            for bb in range(NSTREAM):
                seg = srel_pad[bb, i * SPC:(i + 1) * SPC].reshape(CPC, 128)
                segrel_arr[:, i * 72 + bb * CPC:i * 72 + (bb + 1) * CPC] = seg.T

        gridT_k = np.ascontiguousarray(
            grid[k * GPC:(k + 1) * GPC].T).astype(f8)

        in_maps.append({
            "gridT": gridT_k,
            "w_pack": w_pack,
            "idx": idxw,
            "segrel": segrel_arr.astype(ml_dtypes.bfloat16),
            "iota": iota,
            "bias": bias_rep,
            "eye": eye,
        })
    return in_maps


def assemble(results):
    out = np.empty((NSEG, NCH), np.float32)
    for k in range(NCORE):
        out[k * SEG_PER_CORE:(k + 1) * SEG_PER_CORE] = results[k]["out"]
    return out


_NC_CACHE = {}


def kernel(class_capsules, W, b, point_idx, segment_ids, num_segments):
    """Full-input entry point: shard across 8 NeuronCores, run, reassemble."""
    from concourse.bass_utils import run_bass_kernel_spmd

    in_maps = prep_inputs(np.asarray(class_capsules), np.asarray(W),
                          np.asarray(b), np.asarray(point_idx),
                          np.asarray(segment_ids), int(num_segments))
    if "nc" not in _NC_CACHE:
        _NC_CACHE["nc"] = build_nc()
    res = run_bass_kernel_spmd(_NC_CACHE["nc"], in_maps, list(range(NCORE)))
    return assemble(res.results)
